# revision 1
# baseline (speedup 1.0000x reference)
"""Trainium2 Bass kernel for nn_AutoEnCode1 (dense_mlp, 8 NeuronCores).

Strategy (edge-data-parallel, per the sharding hint):
  - Shard the E=8192 edges across 8 cores (1024 edges each).
  - Host-side sharding prep (data movement / layout only, no FLOPs):
    gather Xi = A[ni], Xj = A[nj] for each core's edge slice, stack to
    Xc = [Xi; Xj] (2048 rows), and ship it in the two layouts the
    TensorEngine needs (natural [e, n] and transposed [n, e]), cast to
    bf16.  Weights are pre-transposed (W1^T [n,h], W2^T [h,m]) and
    replicated.
  - Device-side compute per core (all the FLOPs):
      mm1: H1^T[h,e'] = W1^T-tiles (stationary) x X^T-tiles (moving),
           fused sigmoid(+b1 per-partition bias) on ScalarE.
      layer-1 loss: (H1i-H1j)^2 partition-reduced via ones-matmul,
           sqrt, * label, accumulated.
      mm2: H2[e,m] = H1^T-tiles (stationary) x W2^T-tiles (moving);
           b2 bias added via a K=1 ones (x) b2 matmul into the same
           PSUM accumulation group; fused sigmoid.
      layer-2 losses: (H2i-H2j)^2, (Xi-H2i)^2, (Xj-H2j)^2 reduced over
           the free axis with fused tensor_tensor_reduce on VectorE,
           sqrt, weighted by label / penalty factor, accumulated.
      One f32 partial-loss scalar per core is DMA'd out.
  - Host-side unshard: sum the 8 partial scalars and add the
    data-independent weight regularizer loss_r (a constant w.r.t. the
    edge data; ~0.003% of total FLOPs).
"""

import numpy as np
import ml_dtypes

import concourse.bass as bass
import concourse.tile as tile
from concourse import bacc, mybir
from concourse.bass_utils import run_bass_kernel_spmd

N_CORES = 8
N = 8192          # node-feature dim (= num nodes)
H = 1024          # bottleneck dim
E = 8192          # num edges
PENALTY = 10.0

E_LOC = E // N_CORES      # 1024 edges per core
EP = 2 * E_LOC            # 2048 stacked rows: [Xi; Xj]

BF16 = mybir.dt.bfloat16
FP8 = mybir.dt.float8e4
F32 = mybir.dt.float32
DR = mybir.MatmulPerfMode.DoubleRow
SIG = mybir.ActivationFunctionType.Sigmoid
SQUARE = mybir.ActivationFunctionType.Square
MULT = mybir.AluOpType.mult
ADD = mybir.AluOpType.add
AXX = mybir.AxisListType.X

ts = bass.ts


def build_nc(n=N, h=H, e_loc=E_LOC, phases="ALL"):
    """Build + compile the per-core Bass graph (identical on all cores)."""
    ep = 2 * e_loc
    NT = n // 128        # contraction tiles, layer 1
    HT = h // 128        # h tiles
    ET = ep // 512       # e' panels of 512 (mm1 moving dim)
    EB = e_loc // 128    # edge blocks of 128 per stream ("pairs")
    MT = n // 512        # m panels of 512 (mm2 moving dim)
    EH = e_loc // 512    # e halves of 512 (layer-1 norm)

    nc = bacc.Bacc("TRN2", target_bir_lowering=False, debug=False,
                   num_devices=N_CORES)

    NT2 = n // 256       # DoubleRow contraction chunks, layer 1
    HC = h // 256        # DoubleRow contraction chunks, layer 2
    xt8 = nc.dram_tensor("xt8", [NT2, 128, 2, ep], FP8, kind="ExternalInput")
    xn = nc.dram_tensor("xn", [ep, n], BF16, kind="ExternalInput")
    w1t8 = nc.dram_tensor("w1t8", [NT2, 128, 2, h], FP8,
                          kind="ExternalInput")
    w2t8 = nc.dram_tensor("w2t8", [HC, 128, 2, n], FP8,
                          kind="ExternalInput")
    b1d = nc.dram_tensor("b1", [h], F32, kind="ExternalInput")
    b2d = nc.dram_tensor("b2", [n], BF16, kind="ExternalInput")
    b2f = nc.dram_tensor("b2f", [128, n], BF16, kind="ExternalInput")
    labd = nc.dram_tensor("lab", [e_loc], F32, kind="ExternalInput")
    facd = nc.dram_tensor("fac", [e_loc], F32, kind="ExternalInput")
    outd = nc.dram_tensor("out", [1], F32, kind="ExternalOutput")

    with tile.TileContext(nc) as tc:
        with (
            tc.tile_pool(name="h1", bufs=1) as h1_pool,
            tc.tile_pool(name="misc", bufs=1) as misc,
            tc.tile_pool(name="fin", bufs=1) as fin,
        ):
            # Persistent tiles
            h1all = h1_pool.tile([128, HT, ep], FP8, name="h1all",
                                 tag="h1all")
            b1t = []
            for t in range(HT):
                bt = misc.tile([128, 1], F32, name=f"b1t{t}", tag=f"b1t{t}")
                nc.sync.dma_start(bt[:], b1d.ap()[ts(t, 128)])
                b1t.append(bt)
            lab_f = misc.tile([1, e_loc], F32, name="labf", tag="labf")
            nc.sync.dma_start(lab_f[:], labd.ap()[:])
            labp = []
            facp = []
            for p in range(EB):
                lt = misc.tile([128, 1], F32, name=f"labp{p}", tag=f"labp{p}")
                nc.sync.dma_start(lt[:], labd.ap()[ts(p, 128)])
                labp.append(lt)
                ft = misc.tile([128, 1], F32, name=f"facp{p}", tag=f"facp{p}")
                nc.sync.dma_start(ft[:], facd.ap()[ts(p, 128)])
                facp.append(ft)
            ones_b = misc.tile([1, 128], BF16, name="ones_b", tag="ones_b")
            nc.gpsimd.memset(ones_b[:], 1.0)
            ones_col = misc.tile([128, 1], BF16, name="ones_col", tag="ones_col")
            nc.gpsimd.memset(ones_col[:], 1.0)
            ones_f32 = misc.tile([128, 1], F32, name="ones_f32", tag="ones_f32")
            nc.gpsimd.memset(ones_f32[:], 1.0)
            l1vec = fin.tile([1, e_loc], F32, name="l1vec", tag="l1vec")
            pacc = fin.tile([128, EB], F32, name="pacc", tag="pacc")

            # ---------------- Phase A: layer 1 matmul ----------------
            with (
                tc.tile_pool(name="w1", bufs=1) as w1_pool,
                tc.tile_pool(name="xa", bufs=4) as xa_pool,
                tc.tile_pool(name="psA", bufs=HT, space="PSUM") as psA,
            ):
                w1sb = [w1_pool.tile([128, 2, h], FP8, name=f"w1_{t}",
                                     tag=f"w1_{t}") for t in range(NT2)]
                for e_t in range(ET):
                    ps = [psA.tile([128, 512], F32, name="psA", tag="psA")
                          for _ in range(HT)]
                    for c in range(NT2):
                        if e_t == 0:
                            nc.sync.dma_start(w1sb[c][:], w1t8.ap()[c])
                        x = xa_pool.tile([128, 2, 512], FP8, name="x",
                                         tag="x")
                        nc.sync.dma_start(x[:], xt8.ap()[c][:, :,
                                                           ts(e_t, 512)])
                        for h_t in range(HT):
                            nc.tensor.matmul(ps[h_t][:],
                                             w1sb[c][:, :, ts(h_t, 128)],
                                             x[:],
                                             start=(c == 0),
                                             stop=(c == NT2 - 1),
                                             perf_mode=DR)
                    for h_t in range(HT):
                        nc.scalar.activation(h1all[:, h_t, ts(e_t, 512)],
                                             ps[h_t][:], SIG,
                                             bias=b1t[h_t][:])

            if phases == "A":
                dumm = fin.tile([1, 1], F32, name="dumm", tag="dumm")
                nc.scalar.activation(dumm[:], h1all[0:1, 0, 0:1],
                                     mybir.ActivationFunctionType.Identity)
                nc.sync.dma_start(outd.ap()[:], dumm[0:1, 0:1])

            # ------- Phase A2 + B share psS; A2 overlaps B's start -------
            if phases != "A":
              with (
                tc.tile_pool(name="l1s", bufs=2) as l1s,
                tc.tile_pool(name="psS", bufs=1, space="PSUM") as psS,
            ):
                # layer-1 diff loss: sqrt(sum_h (H1i-H1j)^2) * lab
                for eh in range(EH):
                    l1ps = psS.tile([1, 512], F32, name="l1ps", tag="l1ps")
                    for h_t in range(HT):
                        d = l1s.tile([128, 512], BF16, name="d", tag="d")
                        nc.vector.tensor_sub(
                            d[:],
                            h1all[:, h_t, eh * 512:(eh + 1) * 512],
                            h1all[:, h_t, e_loc + eh * 512:
                                  e_loc + (eh + 1) * 512])
                        d2 = l1s.tile([128, 512], BF16, name="d2", tag="d2")
                        nc.scalar.square(d2[:], d[:])
                        nc.tensor.matmul(l1ps[:], ones_col[:], d2[:],
                                         start=(h_t == 0),
                                         stop=(h_t == HT - 1))
                    l1n = l1s.tile([1, 512], F32, name="l1n", tag="l1n",
                                   bufs=1)
                    nc.scalar.sqrt(l1n[:], l1ps[:])
                    nc.vector.tensor_mul(
                        l1vec[:, eh * 512:(eh + 1) * 512], l1n[:],
                        lab_f[:, eh * 512:(eh + 1) * 512])

                if phases == "A2":
                    dumm = fin.tile([1, 1], F32, name="dumm", tag="dumm")
                    nc.vector.reduce_sum(dumm[:], l1vec[:], axis=AXX)
                    nc.sync.dma_start(outd.ap()[:], dumm[0:1, 0:1])

                # ---------------- Phase B: layer 2 ----------------
                if phases != "A2":
                  with (
                    tc.tile_pool(name="w2", bufs=1) as w2_pool,
                    tc.tile_pool(name="bb2", bufs=1) as bb2,
                    tc.tile_pool(name="xb", bufs=3) as xb_pool,
                    tc.tile_pool(name="h2", bufs=2) as h2_pool,
                    tc.tile_pool(name="dd", bufs=2) as dd_pool,
                    tc.tile_pool(name="acc", bufs=2) as acc_pool,
                    tc.tile_pool(name="psB", bufs=2, space="PSUM") as psB,
                ):
                    w2sb = [w2_pool.tile([128, 2, n], FP8, name=f"w2_{t}",
                                         tag=f"w2_{t}") for t in range(HC)]
                    junk = bb2.tile([128, 512], BF16, name="junk", tag="junk")
                    junkv = bb2.tile([128, 512], BF16, name="junkv",
                                     tag="junkv")

                    for p in range(EB):
                        accd = acc_pool.tile([128, MT], F32, name="accd", tag="accd")
                        accri = acc_pool.tile([128, MT], F32, name="accri", tag="accri")
                        accrj = acc_pool.tile([128, MT], F32, name="accrj", tag="accrj")
                        for m_t in range(MT):
                            psi = psB.tile([128, 512], F32, name="psi", tag="psi")
                            psj = psB.tile([128, 512], F32, name="psj", tag="psj")
                            b2t = xb_pool.tile([128, 512], BF16,
                                               name="b2t", tag="b2t", bufs=2)
                            nc.sync.dma_start(b2t[:],
                                              b2f.ap()[:, ts(m_t, 512)])
                            for cc in range(HC):
                                if p == 0 and m_t == 0:
                                    nc.sync.dma_start(w2sb[cc][:],
                                                      w2t8.ap()[cc])
                                nc.tensor.matmul(
                                    psi[:],
                                    h1all[:, 2 * cc:2 * cc + 2, ts(p, 128)],
                                    w2sb[cc][:, :, ts(m_t, 512)],
                                    start=(cc == 0), stop=(cc == HC - 1),
                                    perf_mode=DR)
                            for cc in range(HC):
                                nc.tensor.matmul(
                                    psj[:],
                                    h1all[:, 2 * cc:2 * cc + 2,
                                          ts(EB + p, 128)],
                                    w2sb[cc][:, :, ts(m_t, 512)],
                                    start=(cc == 0), stop=(cc == HC - 1),
                                    perf_mode=DR)
                            si = dd_pool.tile([128, 512], BF16, name="siB", tag="siB")
                            nc.vector.tensor_add(si[:], psi[:], b2t[:])
                            sj = dd_pool.tile([128, 512], BF16, name="sjB", tag="sjB")
                            nc.vector.tensor_add(sj[:], psj[:], b2t[:])
                            h2i = h2_pool.tile([128, 512], BF16, name="h2i", tag="h2i")
                            nc.scalar.activation(h2i[:], si[:], SIG)
                            h2j = h2_pool.tile([128, 512], BF16, name="h2j", tag="h2j")
                            nc.scalar.activation(h2j[:], sj[:], SIG)

                            xi = xb_pool.tile([128, 512], BF16, name="xi", tag="xi")
                            nc.sync.dma_start(xi[:],
                                              xn.ap()[ts(p, 128),
                                                      ts(m_t, 512)])
                            xj = xb_pool.tile([128, 512], BF16, name="xj", tag="xj")
                            nc.sync.dma_start(xj[:],
                                              xn.ap()[ts(EB + p, 128),
                                                      ts(m_t, 512)])

                            d = dd_pool.tile([128, 512], BF16, name="dB", tag="dB")
                            nc.vector.tensor_sub(d[:], h2i[:], h2j[:])
                            nc.scalar.activation(
                                junk[:], d[:], SQUARE,
                                accum_out=accd[:, m_t:m_t + 1])
                            dri = dd_pool.tile([128, 512], BF16, name="dri", tag="dri")
                            nc.vector.tensor_sub(dri[:], xi[:], h2i[:])
                            nc.vector.scalar_tensor_tensor(
                                junkv[:], dri[:], 0.0, dri[:], ADD, MULT,
                                accum_out=accri[:, m_t:m_t + 1])
                            drj = dd_pool.tile([128, 512], BF16, name="drj", tag="drj")
                            nc.vector.tensor_sub(drj[:], xj[:], h2j[:])
                            nc.vector.scalar_tensor_tensor(
                                junkv[:], drj[:], 0.0, drj[:], ADD, MULT,
                                accum_out=accrj[:, m_t:m_t + 1])

                        # per-pair finalize -> pacc[:, p]
                        sd = acc_pool.tile([128, 1], F32, name="sd", tag="sd")
                        nc.vector.reduce_sum(sd[:], accd[:], axis=AXX)
                        sri = acc_pool.tile([128, 1], F32, name="sri", tag="sri")
                        nc.vector.reduce_sum(sri[:], accri[:], axis=AXX)
                        srj = acc_pool.tile([128, 1], F32, name="srj", tag="srj")
                        nc.vector.reduce_sum(srj[:], accrj[:], axis=AXX)
                        nd = acc_pool.tile([128, 1], F32, name="nd", tag="nd")
                        nc.scalar.sqrt(nd[:], sd[:])
                        nri = acc_pool.tile([128, 1], F32, name="nri", tag="nri")
                        nc.scalar.sqrt(nri[:], sri[:])
                        nrj = acc_pool.tile([128, 1], F32, name="nrj", tag="nrj")
                        nc.scalar.sqrt(nrj[:], srj[:])
                        t1 = acc_pool.tile([128, 1], F32, name="t1", tag="t1")
                        nc.vector.tensor_mul(t1[:], nd[:], labp[p][:])
                        t2 = acc_pool.tile([128, 1], F32, name="t2", tag="t2")
                        nc.vector.tensor_add(t2[:], nri[:], nrj[:])
                        t3 = acc_pool.tile([128, 1], F32, name="t3", tag="t3")
                        nc.vector.tensor_mul(t3[:], t2[:], facp[p][:])
                        nc.vector.tensor_add(pacc[:, p:p + 1],
                                             t1[:], t3[:])

                # -------- final combine (partition+free collapse) --------
                pv = fin.tile([128, 1], F32, name="pv", tag="pv")
                nc.vector.reduce_sum(pv[:], pacc[:], axis=AXX)
                l1sc = fin.tile([1, 1], F32, name="l1sc", tag="l1sc")
                nc.vector.reduce_sum(l1sc[:], l1vec[:], axis=AXX)
                fps = psS.tile([1, 1], F32, name="fps", tag="fps")
                nc.tensor.matmul(fps[:], pv[:], ones_f32[:],
                                 start=True, stop=True)
                tot = fin.tile([1, 1], F32, name="tot", tag="tot")
                nc.vector.tensor_add(tot[:], fps[:], l1sc[:])
                nc.sync.dma_start(outd.ap()[:], tot[0:1, 0:1])

    nc.compile()
    return nc


_NC_CACHE = {}


def _get_nc():
    if "nc" not in _NC_CACHE:
        _NC_CACHE["nc"] = build_nc()
    return _NC_CACHE["nc"]


def make_in_maps(A, W1, b1, W2, b2, edges, labels):
    bf16 = ml_dtypes.bfloat16
    ni = edges[:, 0].astype(np.int64)
    nj = edges[:, 1].astype(np.int64)

    fp8 = ml_dtypes.float8_e4m3
    nn, hh = W1.shape[1], W1.shape[0]
    # interleaved DoubleRow layouts: [chunk, p, plane, out-dim] where
    # contraction row k = 256*chunk + 128*plane + p
    W1T8 = np.ascontiguousarray(
        W1.T.reshape(nn // 256, 2, 128, hh).transpose(0, 2, 1, 3)
    ).astype(fp8)
    W2T8 = np.ascontiguousarray(
        W2.T.reshape(hh // 256, 2, 128, nn).transpose(0, 2, 1, 3)
    ).astype(fp8)
    b1f = b1.astype(np.float32)
    b2bf = b2.astype(bf16)
    b2full = np.ascontiguousarray(np.broadcast_to(b2bf, (128, b2bf.shape[0])))

    in_maps = []
    for c in range(N_CORES):
        sl = slice(c * E_LOC, (c + 1) * E_LOC)
        Xc = np.concatenate([A[ni[sl]], A[nj[sl]]], axis=0)   # [EP, N] f32
        Xcb = Xc.astype(bf16)
        XT8 = np.ascontiguousarray(
            Xc.T.reshape(nn // 256, 2, 128, Xc.shape[0]).transpose(0, 2, 1, 3)
        ).astype(fp8)
        lab = labels[sl].astype(np.float32)
        fac = np.where(lab >= 1.0, np.float32(PENALTY),
                       np.float32(1.0)).astype(np.float32)
        in_maps.append({
            "xt8": XT8, "xn": Xcb, "w1t8": W1T8, "w2t8": W2T8,
            "b1": b1f, "b2": b2bf, "b2f": b2full, "lab": lab, "fac": fac,
        })
    return in_maps


def host_loss_r(W1, b1, W2, b2):
    # Weight regularizer: data-independent constant (per-edge, per-layer
    # sum of weight row norms + bias norm, times E).
    return float(E) * (
        np.linalg.norm(W1, axis=1).sum() + np.linalg.norm(b1)
        + np.linalg.norm(W2, axis=1).sum() + np.linalg.norm(b2)
    )


def kernel(A, W1, b1, W2, b2, edges, labels):
    A = np.asarray(A, dtype=np.float32)
    W1 = np.asarray(W1, dtype=np.float32)
    b1 = np.asarray(b1, dtype=np.float32)
    W2 = np.asarray(W2, dtype=np.float32)
    b2 = np.asarray(b2, dtype=np.float32)
    edges = np.asarray(edges)
    labels = np.asarray(labels)

    in_maps = make_in_maps(A, W1, b1, W2, b2, edges, labels)
    nc = _get_nc()
    res = run_bass_kernel_spmd(nc, in_maps, core_ids=list(range(N_CORES)))
    part = sum(float(res.results[c]["out"][0]) for c in range(N_CORES))
    return np.array(part + host_loss_r(W1, b1, W2, b2), dtype=np.float32)



# revision 2
# speedup vs baseline: 1.1033x; 1.1033x over previous
"""Trainium2 Bass kernel for nn_AutoEnCode1 (dense_mlp, 8 NeuronCores).

Strategy (edge-data-parallel, per the sharding hint):
  - Shard the E=8192 edges across 8 cores (1024 edges each).
  - Host-side sharding prep (data movement / layout only, no FLOPs):
    gather Xi = A[ni], Xj = A[nj] for each core's edge slice, stack to
    Xc = [Xi; Xj] (2048 rows), ship in the layouts the TensorEngine
    needs (fp8 DoubleRow-interleaved [n, e] for mm1; bf16 [e, n] for
    the recon loss).  Weights pre-transposed + fp8, replicated.
  - Device-side compute per core (all the FLOPs):
      mm1: H1^T[h,e'] = W1^T-tiles x X^T-tiles (fp8 DoubleRow), fused
           sigmoid(+b1 per-partition bias) on ScalarE -> fp8 h1all.
      layer-1 loss: (H1i-H1j)^2 partition-reduced via ones-matmul,
           sqrt, * label.
      mm2: H2[e,m] = H1^T-tiles (stationary) x W2^T-tiles (moving),
           b2 bias folded into the same PSUM accumulation group via a
           rank-1 ones x b2 matmul; fused sigmoid on ScalarE.
      layer-2 losses: (H2i-H2j)^2 via ScalarE Square+accum,
           (Xi-H2i)^2 / (Xj-H2j)^2 via VectorE sub + STT mult+accum,
           all reductions riding free accum_out columns; sqrt,
           weighted by label / penalty factor.
      Per-core partials: a [128] per-edge-row loss vector + the
      layer-1 scalar are DMA'd out (129 floats).
  - Host-side unshard: sum the 8x129 partials and add the
    data-independent weight regularizer loss_r.
"""

import numpy as np
import ml_dtypes

import concourse.bass as bass
import concourse.tile as tile
from concourse import bacc, mybir
from concourse.bass_utils import run_bass_kernel_spmd

N_CORES = 8
N = 8192          # node-feature dim (= num nodes)
H = 1024          # bottleneck dim
E = 8192          # num edges
PENALTY = 10.0

E_LOC = E // N_CORES      # 1024 edges per core
EP = 2 * E_LOC            # 2048 stacked rows: [Xi; Xj]

BF16 = mybir.dt.bfloat16
FP8 = mybir.dt.float8e4
F32 = mybir.dt.float32
DR = mybir.MatmulPerfMode.DoubleRow
SIG = mybir.ActivationFunctionType.Sigmoid
SQUARE = mybir.ActivationFunctionType.Square
MULT = mybir.AluOpType.mult
ADD = mybir.AluOpType.add
AXX = mybir.AxisListType.X

ts = bass.ts


def build_nc(n=N, h=H, e_loc=E_LOC, phases="ALL"):
    """Build + compile the per-core Bass graph (identical on all cores)."""
    ep = 2 * e_loc
    HT = h // 128        # h tiles
    ET = ep // 512       # e' panels of 512 (mm1 moving dim)
    EB = e_loc // 128    # edge blocks of 128 per stream ("pairs")
    EH = e_loc // 512    # e halves of 512 (layer-1 norm)
    CH = n // 1024       # phase-B m chunks of 1024 (2 PSUM banks/stream)

    nc = bacc.Bacc("TRN2", target_bir_lowering=False, debug=False,
                   num_devices=N_CORES)

    NT2 = n // 256       # DoubleRow contraction chunks, layer 1
    HC = h // 256        # DoubleRow contraction chunks, layer 2
    xt8 = nc.dram_tensor("xt8", [NT2, 128, 2, ep], FP8, kind="ExternalInput")
    xn = nc.dram_tensor("xn", [ep, n], BF16, kind="ExternalInput")
    w1t8 = nc.dram_tensor("w1t8", [NT2, 128, 2, h], FP8,
                          kind="ExternalInput")
    w2t8 = nc.dram_tensor("w2t8", [HC, 128, 2, n], FP8,
                          kind="ExternalInput")
    b1d = nc.dram_tensor("b1", [h], F32, kind="ExternalInput")
    b2d = nc.dram_tensor("b2", [n], BF16, kind="ExternalInput")
    labd = nc.dram_tensor("lab", [e_loc], F32, kind="ExternalInput")
    facd = nc.dram_tensor("fac", [e_loc], F32, kind="ExternalInput")
    outd = nc.dram_tensor("out", [129], F32, kind="ExternalOutput")

    with tile.TileContext(nc) as tc:
        with (
            tc.tile_pool(name="h1", bufs=1) as h1_pool,
            tc.tile_pool(name="misc", bufs=1) as misc,
            tc.tile_pool(name="fin", bufs=1) as fin,
        ):
            # Persistent tiles
            h1all = h1_pool.tile([128, HT, ep], FP8, name="h1all",
                                 tag="h1all")
            b1t = []
            for t in range(HT):
                bt = misc.tile([128, 1], F32, name=f"b1t{t}", tag=f"b1t{t}")
                nc.sync.dma_start(bt[:], b1d.ap()[ts(t, 128)])
                b1t.append(bt)
            b2sb = misc.tile([1, n], BF16, name="b2sb", tag="b2sb")
            nc.sync.dma_start(b2sb[:], b2d.ap()[:])
            lab_f = misc.tile([1, e_loc], F32, name="labf", tag="labf")
            nc.sync.dma_start(lab_f[:], labd.ap()[:])
            labp = []
            facp = []
            for p in range(EB):
                lt = misc.tile([128, 1], F32, name=f"labp{p}", tag=f"labp{p}")
                nc.sync.dma_start(lt[:], labd.ap()[ts(p, 128)])
                labp.append(lt)
                ft = misc.tile([128, 1], F32, name=f"facp{p}", tag=f"facp{p}")
                nc.sync.dma_start(ft[:], facd.ap()[ts(p, 128)])
                facp.append(ft)
            ones_b = misc.tile([1, 128], BF16, name="ones_b", tag="ones_b")
            nc.gpsimd.memset(ones_b[:], 1.0)
            ones_col = misc.tile([128, 1], BF16, name="ones_col",
                                 tag="ones_col")
            nc.gpsimd.memset(ones_col[:], 1.0)
            l1vec = fin.tile([1, e_loc], F32, name="l1vec", tag="l1vec")
            pacc = fin.tile([128, EB], F32, name="pacc", tag="pacc")

            # ---------------- Phase A: layer 1 matmul ----------------
            with (
                tc.tile_pool(name="w1", bufs=1) as w1_pool,
                tc.tile_pool(name="xa", bufs=4) as xa_pool,
                tc.tile_pool(name="psA", bufs=HT, space="PSUM") as psA,
            ):
                w1sb = [w1_pool.tile([128, 2, h], FP8, name=f"w1_{t}",
                                     tag=f"w1_{t}") for t in range(NT2)]
                for e_t in range(ET):
                    ps = [psA.tile([128, 512], F32, name="psA", tag="psA")
                          for _ in range(HT)]
                    for c in range(NT2):
                        if e_t == 0:
                            nc.sync.dma_start(w1sb[c][:], w1t8.ap()[c])
                        x = xa_pool.tile([128, 2, 512], FP8, name="x",
                                         tag="x")
                        nc.sync.dma_start(x[:], xt8.ap()[c][:, :,
                                                           ts(e_t, 512)])
                        for h_t in range(HT):
                            nc.tensor.matmul(ps[h_t][:],
                                             w1sb[c][:, :, ts(h_t, 128)],
                                             x[:],
                                             start=(c == 0),
                                             stop=(c == NT2 - 1),
                                             perf_mode=DR)
                    for h_t in range(HT):
                        nc.scalar.activation(h1all[:, h_t, ts(e_t, 512)],
                                             ps[h_t][:], SIG,
                                             bias=b1t[h_t][:])

            if phases == "A":
                dumm = fin.tile([1, 1], F32, name="dumm", tag="dumm")
                nc.scalar.activation(dumm[:], h1all[0:1, 0, 0:1],
                                     mybir.ActivationFunctionType.Identity)
                nc.sync.dma_start(outd.ap()[0:1], dumm[0:1, 0:1])

            # ---------------- Phase A2: layer-1 diff loss ----------------
            if phases != "A":
                with (
                    tc.tile_pool(name="l1s", bufs=2) as l1s,
                    tc.tile_pool(name="psS", bufs=1, space="PSUM") as psS,
                ):
                    # sqrt(sum_h (H1i-H1j)^2) * lab
                    for eh in range(EH):
                        l1ps = psS.tile([1, 512], F32, name="l1ps",
                                        tag="l1ps")
                        for h_t in range(HT):
                            d = l1s.tile([128, 512], BF16, name="d", tag="d")
                            nc.vector.tensor_sub(
                                d[:],
                                h1all[:, h_t, eh * 512:(eh + 1) * 512],
                                h1all[:, h_t, e_loc + eh * 512:
                                      e_loc + (eh + 1) * 512])
                            d2 = l1s.tile([128, 512], BF16, name="d2",
                                          tag="d2")
                            nc.vector.scalar_tensor_tensor(
                                d2[:], d[:], 0.0, d[:], ADD, MULT)
                            nc.tensor.matmul(l1ps[:], ones_col[:], d2[:],
                                             start=(h_t == 0),
                                             stop=(h_t == HT - 1))
                        l1n = l1s.tile([1, 512], F32, name="l1n", tag="l1n",
                                       bufs=1)
                        nc.scalar.sqrt(l1n[:], l1ps[:])
                        nc.vector.tensor_mul(
                            l1vec[:, eh * 512:(eh + 1) * 512], l1n[:],
                            lab_f[:, eh * 512:(eh + 1) * 512])
                    l1sc = fin.tile([1, 1], F32, name="l1sc", tag="l1sc")
                    nc.vector.reduce_sum(l1sc[:], l1vec[:], axis=AXX)

                if phases == "A2":
                    nc.sync.dma_start(outd.ap()[128:129], l1sc[0:1, 0:1])

                # ---------------- Phase B: layer 2 ----------------
                if phases != "A2":
                  with (
                    tc.tile_pool(name="w2", bufs=1) as w2_pool,
                    tc.tile_pool(name="xb", bufs=4) as xb_pool,
                    tc.tile_pool(name="h2", bufs=3) as h2_pool,
                    tc.tile_pool(name="dd", bufs=3) as dd_pool,
                    tc.tile_pool(name="acc", bufs=2) as acc_pool,
                    tc.tile_pool(name="psB", bufs=2, space="PSUM") as psB,
                  ):
                    w2sb = [w2_pool.tile([128, 2, n], FP8, name=f"w2_{t}",
                                         tag=f"w2_{t}") for t in range(HC)]
                    junk = misc.tile([128, 1024], BF16, name="junk",
                                     tag="junk")
                    junkv = misc.tile([128, 1024], BF16, name="junkv",
                                      tag="junkv")

                    for p in range(EB):
                        accd = acc_pool.tile([128, CH], F32, name="accd",
                                             tag="accd")
                        accri = acc_pool.tile([128, CH], F32, name="accri",
                                              tag="accri")
                        accrj = acc_pool.tile([128, CH], F32, name="accrj",
                                              tag="accrj")
                        for ch in range(CH):
                            psi = psB.tile([128, 1024], F32, name="psi",
                                           tag="psi")
                            psj = psB.tile([128, 1024], F32, name="psj",
                                           tag="psj")
                            # rank-1 b2 bias matmuls open each accum group
                            for s in range(2):
                                nc.tensor.matmul(
                                    psi[:, ts(s, 512)], ones_b[:],
                                    b2sb[:, ch * 1024 + s * 512:
                                         ch * 1024 + (s + 1) * 512],
                                    start=True, stop=False)
                            for s in range(2):
                                nc.tensor.matmul(
                                    psj[:, ts(s, 512)], ones_b[:],
                                    b2sb[:, ch * 1024 + s * 512:
                                         ch * 1024 + (s + 1) * 512],
                                    start=True, stop=False)
                            for cc in range(HC):
                                if p == 0 and ch == 0:
                                    nc.sync.dma_start(w2sb[cc][:],
                                                      w2t8.ap()[cc])
                                for s in range(2):
                                    nc.tensor.matmul(
                                        psi[:, ts(s, 512)],
                                        h1all[:, 2 * cc:2 * cc + 2,
                                              ts(p, 128)],
                                        w2sb[cc][:, :,
                                                 ch * 1024 + s * 512:
                                                 ch * 1024 + (s + 1) * 512],
                                        start=False, stop=(cc == HC - 1),
                                        perf_mode=DR)
                            for cc in range(HC):
                                for s in range(2):
                                    nc.tensor.matmul(
                                        psj[:, ts(s, 512)],
                                        h1all[:, 2 * cc:2 * cc + 2,
                                              ts(EB + p, 128)],
                                        w2sb[cc][:, :,
                                                 ch * 1024 + s * 512:
                                                 ch * 1024 + (s + 1) * 512],
                                        start=False, stop=(cc == HC - 1),
                                        perf_mode=DR)
                            h2i = h2_pool.tile([128, 1024], BF16, name="h2i",
                                               tag="h2i")
                            nc.scalar.activation(h2i[:], psi[:], SIG)
                            h2j = h2_pool.tile([128, 1024], BF16, name="h2j",
                                               tag="h2j")
                            nc.scalar.activation(h2j[:], psj[:], SIG)

                            xi = xb_pool.tile([128, 1024], BF16, name="xi",
                                              tag="xi")
                            nc.sync.dma_start(xi[:],
                                              xn.ap()[ts(p, 128),
                                                      ts(ch, 1024)])
                            xj = xb_pool.tile([128, 1024], BF16, name="xj",
                                              tag="xj")
                            nc.sync.dma_start(xj[:],
                                              xn.ap()[ts(EB + p, 128),
                                                      ts(ch, 1024)])

                            d = dd_pool.tile([128, 1024], BF16, name="dB",
                                             tag="dB")
                            nc.vector.tensor_sub(d[:], h2i[:], h2j[:])
                            nc.scalar.activation(
                                junk[:], d[:], SQUARE,
                                accum_out=accd[:, ch:ch + 1])
                            dri = dd_pool.tile([128, 1024], BF16, name="dri",
                                               tag="dri")
                            nc.vector.tensor_sub(dri[:], xi[:], h2i[:])
                            nc.vector.scalar_tensor_tensor(
                                junkv[:], dri[:], 0.0, dri[:], ADD, MULT,
                                accum_out=accri[:, ch:ch + 1])
                            drj = dd_pool.tile([128, 1024], BF16, name="drj",
                                               tag="drj")
                            nc.vector.tensor_sub(drj[:], xj[:], h2j[:])
                            nc.vector.scalar_tensor_tensor(
                                junkv[:], drj[:], 0.0, drj[:], ADD, MULT,
                                accum_out=accrj[:, ch:ch + 1])

                        # per-pair finalize -> pacc[:, p]
                        sd = acc_pool.tile([128, 1], F32, name="sd", tag="sd")
                        nc.vector.reduce_sum(sd[:], accd[:], axis=AXX)
                        sri = acc_pool.tile([128, 1], F32, name="sri",
                                            tag="sri")
                        nc.vector.reduce_sum(sri[:], accri[:], axis=AXX)
                        srj = acc_pool.tile([128, 1], F32, name="srj",
                                            tag="srj")
                        nc.vector.reduce_sum(srj[:], accrj[:], axis=AXX)
                        nd = acc_pool.tile([128, 1], F32, name="nd", tag="nd")
                        nc.scalar.sqrt(nd[:], sd[:])
                        nri = acc_pool.tile([128, 1], F32, name="nri",
                                            tag="nri")
                        nc.scalar.sqrt(nri[:], sri[:])
                        nrj = acc_pool.tile([128, 1], F32, name="nrj",
                                            tag="nrj")
                        nc.scalar.sqrt(nrj[:], srj[:])
                        t1 = acc_pool.tile([128, 1], F32, name="t1", tag="t1")
                        nc.vector.tensor_mul(t1[:], nd[:], labp[p][:])
                        t2 = acc_pool.tile([128, 1], F32, name="t2", tag="t2")
                        nc.vector.tensor_add(t2[:], nri[:], nrj[:])
                        t3 = acc_pool.tile([128, 1], F32, name="t3", tag="t3")
                        nc.vector.tensor_mul(t3[:], t2[:], facp[p][:])
                        nc.vector.tensor_add(pacc[:, p:p + 1],
                                             t1[:], t3[:])

                    # -------- outputs: per-row partials + l1 scalar --------
                    pv = fin.tile([128, 1], F32, name="pv", tag="pv")
                    nc.vector.reduce_sum(pv[:], pacc[:], axis=AXX)
                    nc.sync.dma_start(outd.ap()[0:128], pv[:, 0:1])
                    nc.sync.dma_start(outd.ap()[128:129], l1sc[0:1, 0:1])

    nc.compile()
    return nc


_NC_CACHE = {}


def _get_nc():
    if "nc" not in _NC_CACHE:
        _NC_CACHE["nc"] = build_nc()
    return _NC_CACHE["nc"]


def make_in_maps(A, W1, b1, W2, b2, edges, labels):
    bf16 = ml_dtypes.bfloat16
    ni = edges[:, 0].astype(np.int64)
    nj = edges[:, 1].astype(np.int64)

    fp8 = ml_dtypes.float8_e4m3
    nn, hh = W1.shape[1], W1.shape[0]
    # interleaved DoubleRow layouts: [chunk, p, plane, out-dim] where
    # contraction row k = 256*chunk + 128*plane + p
    W1T8 = np.ascontiguousarray(
        W1.T.reshape(nn // 256, 2, 128, hh).transpose(0, 2, 1, 3)
    ).astype(fp8)
    W2T8 = np.ascontiguousarray(
        W2.T.reshape(hh // 256, 2, 128, nn).transpose(0, 2, 1, 3)
    ).astype(fp8)
    b1f = b1.astype(np.float32)
    b2bf = b2.astype(bf16)

    in_maps = []
    for c in range(N_CORES):
        sl = slice(c * E_LOC, (c + 1) * E_LOC)
        Xc = np.concatenate([A[ni[sl]], A[nj[sl]]], axis=0)   # [EP, N] f32
        Xcb = Xc.astype(bf16)
        XT8 = np.ascontiguousarray(
            Xc.T.reshape(nn // 256, 2, 128, Xc.shape[0]).transpose(0, 2, 1, 3)
        ).astype(fp8)
        lab = labels[sl].astype(np.float32)
        fac = np.where(lab >= 1.0, np.float32(PENALTY),
                       np.float32(1.0)).astype(np.float32)
        in_maps.append({
            "xt8": XT8, "xn": Xcb, "w1t8": W1T8, "w2t8": W2T8,
            "b1": b1f, "b2": b2bf, "lab": lab, "fac": fac,
        })
    return in_maps


def host_loss_r(W1, b1, W2, b2):
    # Weight regularizer: data-independent constant (per-edge, per-layer
    # sum of weight row norms + bias norm, times E).
    return float(E) * (
        np.linalg.norm(W1, axis=1).sum() + np.linalg.norm(b1)
        + np.linalg.norm(W2, axis=1).sum() + np.linalg.norm(b2)
    )


def kernel(A, W1, b1, W2, b2, edges, labels):
    A = np.asarray(A, dtype=np.float32)
    W1 = np.asarray(W1, dtype=np.float32)
    b1 = np.asarray(b1, dtype=np.float32)
    W2 = np.asarray(W2, dtype=np.float32)
    b2 = np.asarray(b2, dtype=np.float32)
    edges = np.asarray(edges)
    labels = np.asarray(labels)

    in_maps = make_in_maps(A, W1, b1, W2, b2, edges, labels)
    nc = _get_nc()
    res = run_bass_kernel_spmd(nc, in_maps, core_ids=list(range(N_CORES)))
    part = sum(float(res.results[c]["out"].sum()) for c in range(N_CORES))
    return np.array(part + host_loss_r(W1, b1, W2, b2), dtype=np.float32)
